# revision 28
# baseline (speedup 1.0000x reference)
"""Trainium2 Bass kernel: 4096x4096 valid 5x5 cross-correlation + scalar bias.

Strategy (8 NeuronCores, SPMD):
  - Shard the OUTPUT by columns: core c computes out[:, 512c : 512c+512]
    (core 7's last 4 columns are padding, trimmed after gather). Each core
    reads x rows (padded to 4324), cols [512c, 512c+516) in bf16.
  - On-core: the 5x5 conv runs as PACKED 64x64 tile-position matmuls on the
    TensorEngine. The PE array is addressed as 4 independent 64x64 quadrant
    tiles (tile_position=(64i, 64j)); packed tile-matmuls stream rhs at ~256
    elem/cycle aggregate -- 2x the full-array banded scheme -- and the 64-row
    banded weight blocks waste only 4/64 output rows instead of 4/128.
  - Chunking: chunk = 60 output rows from a 64-row input window. Group = 4
    chunks mapped onto the 4 PE quadrant tiles; supergroup = 2 groups sharing
    one weight load per kernel-column tap dj. PSUM: 2 banks per group -> 4
    supergroups' banks cycle through the 8 banks, so drains overlap matmuls.
  - Output is written SPARSELY to DRAM (each 60-row chunk padded to a 64-row
    strip) so every drain is a full 128-partition copy and every output DMA
    is one contiguous 128-row block; the host strips the 4-row pads for free.
  - PSUM accumulation fp32; bias fused into the PSUM->SBUF drain (split
    across VectorE and ScalarE); drains emit bf16 to halve output DMA bytes.
"""
import os

os.environ.setdefault("MYCRO_LOCAL_CACHE", "1")

import numpy as np

import concourse.bass as bass
import concourse.bacc as bacc
import concourse.tile as tile
import concourse.mybir as mybir
from concourse import bass_utils
from concourse.bass import AP

H, W = 4096, 4096
KH, KW = 5, 5
OH, OW = H - KH + 1, W - KW + 1          # 4092, 4092
NCORES = 8
COLS = 512                               # output cols per core
XC = COLS + KW - 1                       # 516 input cols per core
CH = 60                                  # valid output rows per chunk
CIN = 64                                 # input rows per chunk (CH + KH - 1)
NCH = 72                                 # chunks (69 real, 3 pad)
NGRP = NCH // 4                          # 18 groups of 4 chunks
NSG = NGRP // 2                          # 9 supergroups of 2 groups
XROWS = CH * (NCH - 1) + CIN             # 4324 padded input rows
OROWS = CIN * NCH                        # 4608 sparse output rows

_compiled = None
TRACE = False            # test harness can flip this for neuron-profile timing
LAST_EXEC_NS = None

# ring rotation for the 36 output DMAs (gpsimd SWDGE + both HWDGE rings)
OUT_RING = ["scalar", "sync", "gpsimd"]


def _build():
    nc = bacc.Bacc("TRN2", target_bir_lowering=False, debug=False,
                   num_devices=NCORES)
    mdt = mybir.dt.bfloat16

    x_dram = nc.dram_tensor("xs", (XROWS, XC), mdt, kind="ExternalInput")
    w_dram = nc.dram_tensor("wmat", (128, KW * 64), mdt,
                            kind="ExternalInput")
    bias_dram = nc.dram_tensor("biast", (128, 1), mybir.dt.float32,
                               kind="ExternalInput")
    out_dram = nc.dram_tensor("out", (OROWS, COLS), mybir.dt.bfloat16,
                              kind="ExternalOutput")

    engs = lambda: {"scalar": nc.scalar, "sync": nc.sync, "gpsimd": nc.gpsimd}

    with tile.TileContext(nc) as tc:
        with (
            tc.tile_pool(name="const", bufs=1) as cpool,
            tc.tile_pool(name="xg", bufs=5) as xpool,
            tc.tile_pool(name="stage", bufs=6) as spool,
            tc.tile_pool(name="psum", bufs=4, space=bass.MemorySpace.PSUM) as ppool,
        ):
            wt = cpool.tile([128, KW * 64], mdt)
            biast = cpool.tile([128, 1], mybir.dt.float32)
            junk = cpool.tile([128, COLS], mdt)
            nc.sync.dma_start(wt[:], w_dram.ap())
            nc.scalar.dma_start(biast[:], bias_dram.ap())

            # HAM warmup: memset a junk tile on-chip (no DMA wait), then run
            # full-array matmuls on it so the PE clock gate is already 8/8
            # when the first real matmul issues (~10us in). The junk PSUM
            # tile joins the "ps" rotation; real MMs overwrite via start=1.
            nc.gpsimd.memset(junk[:], 0)
            wps = ppool.tile([128, COLS], mybir.dt.float32, name="warm",
                             tag="ps")
            for _ in range(8):
                nc.tensor.matmul(wps[:], junk[:, 0:128], junk[:],
                                 start=True, stop=True)

            # Input: one [128, 4*516] tile per SUPERGROUP; partition strip i
            # (64 rows) holds chunks 8s+4i .. 8s+4i+3 side by side (the
            # DRAM-side AP overlaps rows by 4, the chunk halo). Two 264KB
            # DMAs per supergroup: HWDGE ring issue cost is flat ~600ns per
            # dma_start, so bigger transfers keep the input stream ahead of
            # the PE. Emitted just-in-time so ring FIFOs stay interleaved.
            xgs = [None] * NSG

            def emit_inputs(s):
                if s >= NSG:
                    return
                xt = xpool.tile([128, 4 * XC], mdt, name=f"xg{s}", tag="xg")
                for i in range(2):
                    r0 = CH * (8 * s + 4 * i)
                    src = AP(tensor=x_dram, offset=r0 * XC,
                             ap=[[XC, CIN], [CH * XC, 4], [1, XC]])
                    dst = xt[64 * i:64 * i + 64, :].rearrange(
                        "p (q c) -> p q c", q=4)
                    ring = nc.sync if i == 0 else nc.scalar
                    ring.dma_start(dst, src)
                xgs[s] = xt

            emit_inputs(0)
            emit_inputs(1)
            emit_inputs(2)

            psall = {}
            state = {"od": 0}

            def emit_drains(s):
                # drains+outputs for supergroup s, emitted one iteration
                # late so output DMAs reach the ring FIFO with their drain
                # semaphores already fired (no head-of-line blocking of the
                # input stream behind a drain-wait). One drain + one 256KB
                # output DMA per 2-bank PSUM tile: HWDGE issue cost is flat
                # per dma_start, so fewer/bigger wins.
                last = s == NSG - 1
                for gi in range(2):
                    stg = spool.tile([128, 2 * COLS], mybir.dt.bfloat16,
                                     name=f"st{s}_{gi}", tag="st")
                    if gi == 0:
                        nc.vector.tensor_scalar_add(stg[:], psall[(s, gi)][:],
                                                    biast[:])
                    else:
                        nc.scalar.activation(
                            stg[:], psall[(s, gi)][:],
                            mybir.ActivationFunctionType.Identity,
                            bias=biast[:])
                    # stage cols [512i : 512i+512] -> DRAM rows
                    # [64*(8s+4i+2gi), +128): 3D AP, i is the outer dim on
                    # both sides
                    r0 = CIN * (8 * s + 2 * gi)
                    dst = AP(tensor=out_dram, offset=r0 * COLS,
                             ap=[[COLS, 128], [4 * CIN * COLS, 2], [1, COLS]])
                    src = stg[:].rearrange("p (i c) -> p i c", i=2)
                    if last:
                        ring = engs()["sync" if gi == 0 else "scalar"]
                    else:
                        ring = engs()[OUT_RING[state["od"] % len(OUT_RING)]]
                    state["od"] += 1
                    ring.dma_start(dst, src)

            for s in range(NSG):
                if s + 3 <= NSG - 1:
                    emit_inputs(s + 3)
                for gi in range(2):
                    psall[(s, gi)] = ppool.tile(
                        [128, 2 * COLS], mybir.dt.float32,
                        name=f"ps{s}_{gi}", tag="ps")
                # weight-stationary: dj outer, both bank-pairs inside share
                # the 4 quadrant weight loads for this tap. PSUM tile gi,
                # bank-half i, col-group j holds chunk 8s + 4i + 2gi + j.
                for dj in range(KW):
                    for gi in range(2):
                        for i in range(2):
                            for j in range(2):
                                q = 2 * gi + j
                                nc.tensor.matmul(
                                    psall[(s, gi)][64 * j:64 * j + 64,
                                                   COLS * i:COLS * i + COLS],
                                    wt[64 * i:64 * i + 64,
                                       64 * dj:64 * dj + 64],
                                    xgs[s][64 * i:64 * i + 64,
                                           XC * q + dj:XC * q + dj + COLS],
                                    start=(dj == 0),
                                    stop=(dj == KW - 1 and j == 1),
                                    tile_position=(64 * i, 64 * j),
                                )
                if s >= 1:
                    emit_drains(s - 1)
            emit_drains(NSG - 1)

    nc.compile()
    return nc


def _banded(weight: np.ndarray) -> np.ndarray:
    """[128, 5*64]: strip i (64 rows) holds the five 64x64 banded blocks
    B_dj[k, m] = w[k-m, dj], identical in both strips."""
    ball = np.zeros((128, KW * 64), dtype=np.float32)
    for i in range(2):
        for dj in range(KW):
            for di in range(KH):
                m = np.arange(64 - di)
                ball[64 * i + m + di, 64 * dj + m] = weight[di, dj]
    return ball


def kernel(x: np.ndarray, weight: np.ndarray, bias: np.ndarray) -> np.ndarray:
    global _compiled
    import ml_dtypes
    x = np.asarray(x, dtype=np.float32)
    weight = np.asarray(weight, dtype=np.float32)
    bias = np.asarray(bias, dtype=np.float32)

    if _compiled is None:
        _compiled = _build()
    nc = _compiled

    xpad = np.zeros((XROWS, NCORES * COLS + KW - 1), dtype=np.float32)
    xpad[:H, :W] = x
    xpad = xpad.astype(ml_dtypes.bfloat16)
    ball = _banded(weight).astype(ml_dtypes.bfloat16)
    bias_col = np.full((128, 1), bias[0], dtype=np.float32)

    in_maps = []
    for c in range(NCORES):
        in_maps.append({
            "xs": np.ascontiguousarray(xpad[:, COLS * c: COLS * c + XC]),
            "wmat": ball,
            "biast": bias_col,
        })

    res = bass_utils.run_bass_kernel_spmd(nc, in_maps,
                                          core_ids=list(range(NCORES)),
                                          trace=TRACE)
    global LAST_EXEC_NS
    LAST_EXEC_NS = res.exec_time_ns
    cores = []
    for c in range(NCORES):
        o = np.asarray(res.results[c]["out"], dtype=np.float32)
        o = o.reshape(NCH, CIN, COLS)[:, :CH, :].reshape(NCH * CH, COLS)
        cores.append(o[:OH])
    out = np.hstack(cores)
    return np.ascontiguousarray(out[:, :OW])
